# revision 1
# baseline (speedup 1.0000x reference)
"""AttMaxPool2D (2x2 softmax-attention pooling) Trainium2 Bass kernel.

Problem: x [16, 224, 224, 128] f32 NHWC -> out [16, 112, 112, 128]
  patches = 2x2 non-overlapping windows; out = sum(p * softmax(p, axis=window)).

Sharding: pure data parallel over batch: 8 cores x 2 examples each.

Per-core layout: partition dim = flattened output row (b_loc*112+ho), free dim
= segments of the input row-pair.  Each chunk loads the even row segment and
the odd row segment (fully contiguous per partition -> 2-dim DMA APs with
4KB-contiguous descriptors), computes exp on ACT, then the softmax-weighted
window sum on DVE:
  out = (A*eA + B*eB + C*eC + D*eD) / (eA+eB+eC+eD)
where A,B = (even row, even/odd col), C,D = (odd row, even/odd col).
"""

import os
from contextlib import ExitStack

import numpy as np

import concourse.bass as bass
import concourse.mybir as mybir
import concourse.tile as tile

F32 = mybir.dt.float32

# Full problem shape (hardcoded per contract).
B, H, W, C = 16, 224, 224, 128
N_CORES = 8
B_LOC = B // N_CORES


def _legalize_waits(nc, max_waits=1):
    """This walrus build's ISA structs accept a single sync-wait command per
    instruction, but Tile's wait emission (not transitively minimal) can leave
    2+ waits.  Two-step fix, semantics-preserving:
      1. prune a wait when it is provably dominated through a kept wait
         (some instruction on the kept wait's engine proc, at/before the kept
         wait value, itself directly waits on the dropped semaphore at >= the
         dropped value);
      2. hoist any remaining extras onto same-engine NoOp instructions
         inserted immediately before (sequencer program order preserves the
         blocking semantics)."""
    import bass_rust
    from concourse.tile_scheduler import PROC_NAME_TO_IDX

    f = nc.m.functions[0]
    insts = [i for b in f.blocks for i in b.instructions]

    def pidx(ant_name):
        return PROC_NAME_TO_IDX[ant_name.rsplit("_", 1)[0]]

    by_proc = {}
    for i in insts:
        p = getattr(i, "bass_scheduled_proc", None)
        t = getattr(i, "bass_scheduled_tick", None)
        if p is None or t is None:
            continue
        by_proc.setdefault(p, []).append((t, i))
    for v in by_proc.values():
        v.sort(key=lambda x: x[0])

    def direct_waits(j):
        si = j.sync_info
        out = {}
        for w in si.on_wait if si else []:
            k = pidx(w.ant_name)
            out[k] = max(out.get(k, -1), w.wait_value)
        return out

    engine_procs = {v for k, v in PROC_NAME_TO_IDX.items()
                    if not k.startswith(("DMAHW", "DMASW", "Collectives"))}

    nop_ctr = [0]
    for b in f.blocks:
        new_insts = []
        for i in b.instructions:
            si = i.sync_info
            if not si or len(si.on_wait) <= max_waits:
                new_insts.append(i)
                continue
            # dedupe per-sem (keep max value)
            best = {}
            for w in si.on_wait:
                k = (w.sync_type, w.id)
                if k not in best or w.wait_value > best[k].wait_value:
                    best[k] = w
            kept = list(best.values())
            # drop same-proc self-waits: an engine instruction waiting on its
            # own proc's semaphore for a tick strictly below its own scheduled
            # tick is guaranteed by program order (the engine runs serially);
            # keeping it only stalls on the ~1us deferred sem-write of the
            # predecessor.
            own_p = getattr(i, "bass_scheduled_proc", None)
            own_t = getattr(i, "bass_scheduled_tick", None)
            if own_p is not None and own_t is not None and i.opcode != "DMACopy":
                kept = [w for w in kept
                        if not (pidx(w.ant_name) == own_p
                                and w.wait_value < own_t)]
            # step 1: transitive pruning
            for wd in list(kept):
                if len(kept) <= max_waits:
                    break
                wd_p, wd_v = pidx(wd.ant_name), wd.wait_value
                ok = False
                for via in kept:
                    if via is wd:
                        continue
                    via_p, via_v = pidx(via.ant_name), via.wait_value
                    if via_p not in engine_procs:
                        continue
                    for t, j in by_proc.get(via_p, []):
                        if t > via_v:
                            break
                        if direct_waits(j).get(wd_p, -1) >= wd_v:
                            ok = True
                            break
                    if ok:
                        break
                if ok:
                    kept.remove(wd)
            # step 2: hoist extras onto preceding same-engine NoOps
            while len(kept) > max_waits:
                w = kept.pop(0)
                nop = mybir.InstNoOp(name=f"I-waitnop-{nop_ctr[0]}", ins=[], outs=[])
                nop_ctr[0] += 1
                nop.engine = i.engine
                nop.sync_info = bass_rust.SyncInfo(on_wait=[w], on_update=[])
                new_insts.append(nop)
            si.on_wait = kept
            new_insts.append(i)
        b.instructions = new_insts
    return nc


def build_kernel(b_loc=B_LOC, h=H, w=W, c=C, f=2048, legalize=True):
    """Emit the per-core kernel. f = input-row segment length (elems) per chunk."""
    ho, wo = h // 2, w // 2
    rowlen = w * c          # elems per input row
    outrow = wo * c         # elems per output row
    rp = b_loc * ho         # total output rows in this shard
    assert rowlen % f == 0
    n_seg = rowlen // f
    g = f // 2              # output elems per partition per chunk
    q = f // (2 * c)        # pixel-pairs per segment

    nc = bass.Bass()
    x = nc.declare_dram_parameter("x", [b_loc, h, w, c], F32, isOutput=False)
    y = nc.declare_dram_parameter("y", [b_loc, ho, wo, c], F32, isOutput=True)

    # [rp, parity(2), rowlen]: row-pairs across the whole shard (batch rows
    # are contiguous so (b h) flattens seamlessly).
    xv = x[:].rearrange("b h w c -> (b h) (w c)").rearrange(
        "(hp par) f -> hp par f", par=2
    )
    yv = y[:].rearrange("b h w c -> (b h) (w c)")  # [rp, outrow]

    # partition blocks over output rows
    blocks = []
    p0 = 0
    while p0 < rp:
        pn = min(128, rp - p0)
        blocks.append((p0, pn))
        p0 += pn

    with ExitStack() as ctx:
        tc = ctx.enter_context(tile.TileContext(nc))
        iop = ctx.enter_context(tc.tile_pool(name="io", bufs=3))
        epp = ctx.enter_context(tc.tile_pool(name="ex", bufs=2))
        tmp = ctx.enter_context(tc.tile_pool(name="tmp", bufs=2))
        outp = ctx.enter_context(tc.tile_pool(name="outp", bufs=1))
        out_ctr = [0]

        mul = mybir.AluOpType.mult
        add = mybir.AluOpType.add

        # prev-chunk state for the software-pipelined division tail:
        # (s1v, rv, n1v, dest-slice, q_l, g_l) of chunk k-1 is finished while
        # chunk k's product ops run, so every dependent pair (t->u->out) is
        # separated by an independent op and pays no DVE DRAIN bubble.
        prev = None

        def emit_tail(st, step):
            s1v_p, rv_p, n1v_p, dst, q_l, g_l = st[:6]
            pn_l = s1v_p.shape[0]
            if step == 0:
                t = tmp.tile([pn_l, g_l], F32, name="t", tag="t")
                st.append(t[:].rearrange("p (q c) -> p q c", q=q_l, c=c))
                nc.vector.tensor_tensor(st[6], s1v_p, rv_p, mul)
            elif step == 1:
                u = tmp.tile([pn_l, g_l], F32, name="u", tag="u")
                st.append(u[:].rearrange("p (q c) -> p q c", q=q_l, c=c))
                nc.vector.scalar_tensor_tensor(
                    st[7], st[6], 2.0, rv_p, mybir.AluOpType.subtract, mul
                )
            else:
                tag = f"outt{out_ctr[0] % 6}"
                out_ctr[0] += 1
                outt = outp.tile([pn_l, g_l], F32, name=tag, tag=tag)
                outtv = outt[:].rearrange("p (q c) -> p q c", q=q_l, c=c)
                nc.vector.scalar_tensor_tensor(outtv, n1v_p, -1.0, st[7], mul, mul)
                nc.sync.dma_start(dst, outt[:])

        for bi, (p0, pn) in enumerate(blocks):
            # split the very first chunk into quarter segments so the first
            # DVE op starts after a quarter-size DMA+exp (pipeline fill)
            if bi == 0 and f % (4 * 2 * c) == 0:
                seglens = [f // 4] * 4 + [f] * (n_seg - 1)
            else:
                seglens = [f] * n_seg
            off = 0
            for fl in seglens:
                ql = fl // (2 * c)
                gl = fl // 2
                xin = iop.tile([pn, 2 * f], F32, name="xin", tag="xin")
                xin3 = xin[:, 0:2 * fl].rearrange("p (par f) -> p par f", par=2)
                # issue input DMA from the ACT sequencer: the exp's WAR/RAW
                # edges become same-engine (no extra sem waits on the DMA)
                nc.scalar.dma_start(xin3, xv[p0:p0 + pn, :, off:off + fl])

                ex = epp.tile([pn, 2 * f], F32, name="ex", tag="ex")
                nc.scalar.activation(ex[:, 0:2 * fl], xin[:, 0:2 * fl],
                                     mybir.ActivationFunctionType.Exp)

                def quad(t):
                    v = t[:, 0:2 * fl].rearrange(
                        "p (half q two c) -> p half q two c",
                        half=2, q=ql, two=2, c=c,
                    )
                    return (v[:, 0, :, 0, :], v[:, 0, :, 1, :],
                            v[:, 1, :, 0, :], v[:, 1, :, 1, :])

                A, Bv, Cv, Dv = quad(xin)
                EA, EB, EC, ED = quad(ex)

                def t3(tag):
                    t = tmp.tile([pn, gl], F32, name=tag, tag=tag)
                    return t, t[:].rearrange("p (q c) -> p q c", q=ql, c=c)

                # s-sum first so the ACT Ln/Exp reciprocal seed overlaps the
                # product chain; accumulations distance-separated from their
                # producers to dodge the per-op DRAIN bubble.
                s1, s1v = t3("s1")
                nc.vector.tensor_tensor(s1v, EA, EB, add)
                n1, n1v = t3("n1")
                nc.vector.tensor_tensor(n1v, A, EA, mul)
                s2, s2v = t3("s2")
                nc.vector.tensor_tensor(s2v, EC, ED, add)
                n2, n2v = t3("n2")
                nc.vector.tensor_tensor(n2v, Bv, EB, mul)
                nc.vector.tensor_tensor(s1v, s1v, s2v, add)

                # 1/s: ACT seed r = exp(-ln(s)) (Ln+Exp share one table set;
                # keeps the ~6 cyc/elem iterative divide off DVE), then one
                # DVE Newton step (in the next chunk's tail) for fp32
                # accuracy:  u = (s*r - 2)*r = -r';  out = (n * -1)*u = n*r'
                lns, _ = t3("lns")
                nc.scalar.activation(lns[:], s1[:], mybir.ActivationFunctionType.Ln)
                r, rv = t3("r")
                nc.scalar.activation(r[:], lns[:], mybir.ActivationFunctionType.Exp,
                                     scale=-1.0)

                n3, n3v = t3("n3")
                nc.vector.tensor_tensor(n3v, Cv, EC, mul)
                if prev is not None:
                    emit_tail(prev, 0)
                n4, n4v = t3("n4")
                nc.vector.tensor_tensor(n4v, Dv, ED, mul)
                if prev is not None:
                    emit_tail(prev, 1)
                nc.vector.tensor_tensor(n1v, n1v, n2v, add)
                if prev is not None:
                    emit_tail(prev, 2)
                nc.vector.tensor_tensor(n3v, n3v, n4v, add)
                nc.vector.tensor_tensor(n1v, n1v, n3v, add)

                prev = [s1v, rv, n1v,
                        yv[p0:p0 + pn, off // 2:off // 2 + gl], ql, gl]
                off += fl

        for step in range(3):
            emit_tail(prev, step)

    return _legalize_waits(nc) if legalize else nc


def kernel(**inputs) -> np.ndarray:
    from concourse.bass_utils import run_bass_kernel_spmd

    x = inputs["x"]
    assert x.shape == (B, H, W, C) and x.dtype == np.float32
    nc = build_kernel()
    shards = x.reshape(N_CORES, B_LOC, H, W, C)
    in_maps = [{"x": np.ascontiguousarray(shards[i])} for i in range(N_CORES)]
    res = run_bass_kernel_spmd(nc, in_maps, list(range(N_CORES)))
    return np.concatenate([r["y"] for r in res.results], axis=0)


if __name__ == "__main__":
    # Small-shape CoreSim validation (no hardware).
    from concourse.bass_interp import CoreSim

    b_loc, h, w, c, f = 1, 8, 16, 128, 1024
    nc = build_kernel(b_loc, h, w, c, f, legalize=False)
    rng = np.random.default_rng(0)
    xs = rng.standard_normal((b_loc, h, w, c), dtype=np.float32)

    sim = CoreSim(nc)
    sim.tensor("x")[:] = xs
    sim.simulate()
    got = sim.tensor("y").copy()

    xd = xs.astype(np.float64)
    p = xd.reshape(b_loc, h // 2, 2, w // 2, 2, c).transpose(0, 1, 3, 2, 4, 5)
    p = p.reshape(b_loc, h // 2, w // 2, 4, c)
    e = np.exp(p - p.max(axis=3, keepdims=True))
    ref = (p * e).sum(axis=3) / e.sum(axis=3)
    err = np.abs(got - ref).max() / np.abs(ref).max()
    print("scale-rel err:", err, "max abs err:", np.abs(got - ref).max())
    assert err < 1e-5, "sim mismatch"
    print("SIM OK")



# revision 9
# speedup vs baseline: 1.9046x; 1.9046x over previous
"""AttMaxPool2D (2x2 softmax-attention pooling) Trainium2 Bass kernel.

Problem: x [16, 224, 224, 128] f32 NHWC -> out [16, 112, 112, 128]
  patches = 2x2 non-overlapping windows; out = sum(p * softmax(p, axis=window)).

Sharding: pure data parallel over batch: 8 cores x 2 examples each.

Per-core layout: each SBUF partition owns a QUARTER of one output-row-pair
(224 row-pairs x 4 quarters = 896 units = 7 full blocks of 128 partitions, so
no idle lanes).  Free dim = segments of the input row-pair quarter; the even
and odd input row segments are loaded contiguously per partition.

Compute (tolerance gate is 2e-2, so bf16 intermediates are fine and give the
DVE its 2x packed mode):
  ACT: E = exp(x)               f32 -> bf16, full chunk
       xb = copy(x even row)    f32 -> bf16 (the A,B window elems; C,D stay
                                f32 and their products run at DVE 1x -- this
                                balances ACT vs DVE busy time)
       r = exp(-ln(s))          deferred one chunk so ACT never waits on DVE
  DVE (all tensor_tensor, bf16 in/out = 2x mode unless noted):
       s = (EA+EB) + (EC+ED)
       m1 = A*EA  m2 = B*EB     (bf16 2x)
       m3 = C*EC  m4 = D*ED     (f32 x bf16, 1x)
       n = (m1+m2) + (m3+m4)
       out = n * r              (bf16; widened to f32 by the store DMA)
  Store: SWDGE (gpsimd) DMA with bf16->f32 cast.
"""

import os
from contextlib import ExitStack

import numpy as np

import concourse.bass as bass
import concourse.mybir as mybir
import concourse.tile as tile

F32 = mybir.dt.float32
BF16 = mybir.dt.bfloat16

# Full problem shape (hardcoded per contract).
B, H, W, C = 16, 224, 224, 128
N_CORES = 8
B_LOC = B // N_CORES
QT = 4  # quarters per row-pair: 224 row-pairs * 4 = 896 = 7 * 128 lanes


def _legalize_waits(nc, max_waits=1):
    """This walrus build's ISA structs accept a single sync-wait command per
    instruction, but Tile's wait emission (not transitively minimal) can leave
    2+ waits.  Two-step fix, semantics-preserving:
      1. prune a wait when it is provably dominated through a kept wait
         (some instruction on the kept wait's engine proc, at/before the kept
         wait value, itself directly waits on the dropped semaphore at >= the
         dropped value);
      2. hoist any remaining extras onto same-engine NoOp instructions
         inserted immediately before (sequencer program order preserves the
         blocking semantics)."""
    import bass_rust
    from concourse.tile_scheduler import PROC_NAME_TO_IDX

    f = nc.m.functions[0]
    insts = [i for b in f.blocks for i in b.instructions]

    def pidx(ant_name):
        return PROC_NAME_TO_IDX[ant_name.rsplit("_", 1)[0]]

    by_proc = {}
    for i in insts:
        p = getattr(i, "bass_scheduled_proc", None)
        t = getattr(i, "bass_scheduled_tick", None)
        if p is None or t is None:
            continue
        by_proc.setdefault(p, []).append((t, i))
    for v in by_proc.values():
        v.sort(key=lambda x: x[0])

    def direct_waits(j):
        si = j.sync_info
        out = {}
        for w in si.on_wait if si else []:
            k = pidx(w.ant_name)
            out[k] = max(out.get(k, -1), w.wait_value)
        return out

    engine_procs = {v for k, v in PROC_NAME_TO_IDX.items()
                    if not k.startswith(("DMAHW", "DMASW", "Collectives"))}

    nop_ctr = [0]
    for b in f.blocks:
        new_insts = []
        for i in b.instructions:
            si = i.sync_info
            if not si or len(si.on_wait) <= max_waits:
                new_insts.append(i)
                continue
            # dedupe per-sem (keep max value)
            best = {}
            for w in si.on_wait:
                k = (w.sync_type, w.id)
                if k not in best or w.wait_value > best[k].wait_value:
                    best[k] = w
            kept = list(best.values())
            # drop same-proc self-waits: an engine instruction waiting on its
            # own proc's semaphore for a tick strictly below its own scheduled
            # tick is guaranteed by program order (the engine runs serially);
            # keeping it only stalls on the ~1us deferred sem-write of the
            # predecessor.
            own_p = getattr(i, "bass_scheduled_proc", None)
            own_t = getattr(i, "bass_scheduled_tick", None)
            if own_p is not None and own_t is not None and i.opcode != "DMACopy":
                kept = [w for w in kept
                        if not (pidx(w.ant_name) == own_p
                                and w.wait_value < own_t)]
            # step 1: transitive pruning
            for wd in list(kept):
                if len(kept) <= max_waits:
                    break
                wd_p, wd_v = pidx(wd.ant_name), wd.wait_value
                ok = False
                for via in kept:
                    if via is wd:
                        continue
                    via_p, via_v = pidx(via.ant_name), via.wait_value
                    if via_p not in engine_procs:
                        continue
                    for t, j in by_proc.get(via_p, []):
                        if t > via_v:
                            break
                        if direct_waits(j).get(wd_p, -1) >= wd_v:
                            ok = True
                            break
                    if ok:
                        break
                if ok:
                    kept.remove(wd)
            # step 2: hoist extras onto preceding same-engine NoOps
            while len(kept) > max_waits:
                w = kept.pop(0)
                nop = mybir.InstNoOp(name=f"I-waitnop-{nop_ctr[0]}", ins=[], outs=[])
                nop_ctr[0] += 1
                nop.engine = i.engine
                nop.sync_info = bass_rust.SyncInfo(on_wait=[w], on_update=[])
                new_insts.append(nop)
            si.on_wait = kept
            new_insts.append(i)
        b.instructions = new_insts
    return nc


def build_kernel(b_loc=B_LOC, h=H, w=W, c=C, f=3584, qt=QT, legalize=True):
    """Emit the per-core kernel. f = input-row-quarter segment len per chunk."""
    ho = h // 2
    rowlen = w * c            # elems per input row
    qrow = rowlen // qt       # input elems per parity per lane-unit
    hp = b_loc * ho           # row-pairs in this shard
    assert qrow % f == 0 and f % (2 * c) == 0
    n_seg = qrow // f
    hp_pb = 32 if hp % 32 == 0 else hp   # row-pairs per partition block
    assert hp % hp_pb == 0
    pn = hp_pb * qt           # partitions per block
    assert pn <= 128
    n_blocks = hp // hp_pb

    nc = bass.Bass()
    x = nc.declare_dram_parameter("x", [b_loc, h, w, c], F32, isOutput=False)
    y = nc.declare_dram_parameter("y", [b_loc, ho, w // 2, c], F32, isOutput=True)

    # x viewed as [par(2), hp, qt, qrow]: batch rows are contiguous so (b h)
    # flattens seamlessly; partition p = (hp_local, qt).  par is outermost so
    # each chunk loads with two 3-dim DMAs (DMA APs are capped at 3 dims).
    xq = (
        x[:]
        .rearrange("b h w c -> (b h) (w c)")
        .rearrange("(hp par) f -> hp par f", par=2)
        .rearrange("hp par (qt s) -> par hp qt s", qt=qt)
    )
    # y viewed as [hp, qt, qrow/2]
    yq = (
        y[:]
        .rearrange("b h w c -> (b h) (w c)")
        .rearrange("hp (qt s) -> hp qt s", qt=qt)
    )

    mul = mybir.AluOpType.mult
    add = mybir.AluOpType.add

    with ExitStack() as ctx:
        tc = ctx.enter_context(tile.TileContext(nc))
        iop = ctx.enter_context(tc.tile_pool(name="io", bufs=2))
        epp = ctx.enter_context(tc.tile_pool(name="ex", bufs=2))
        tmp = ctx.enter_context(tc.tile_pool(name="tmp", bufs=1))
        dfr = ctx.enter_context(tc.tile_pool(name="dfr", bufs=2))

        # chunk list: (block, in-offset, seg-len).  The first chunk is split
        # small so the first DVE op starts after a short DMA+exp pipeline fill.
        chunks = []
        for bi in range(n_blocks):
            if bi == 0 and f == 3584:
                seglens = [512, 1280, 1792] + [f] * (n_seg - 1)
            else:
                seglens = [f] * n_seg
            off = 0
            for fl in seglens:
                chunks.append((bi, off, fl))
                off += fl

        def load(k):
            bi, off, fl = chunks[k]
            hp0 = bi * hp_pb
            xin = iop.tile([pn, 2 * f], F32, name="xin", tag="xin")
            for par in range(2):
                nc.scalar.dma_start(
                    xin[:, par * fl:(par + 1) * fl],
                    xq[par, hp0:hp0 + hp_pb, :, off:off + fl],
                )
            return xin

        def quad(t, fl, dq=1):
            # [p, par, q, 2, c] window views of a [p, 2*fl] chunk tile
            v = t[:, 0:2 * fl // dq].rearrange(
                "p (par q two c) -> p par q two c",
                par=2 // dq, q=fl // (2 * c), two=2, c=c,
            )
            if dq == 2:
                return v[:, 0, :, 0, :], v[:, 0, :, 1, :]
            return (v[:, 0, :, 0, :], v[:, 0, :, 1, :],
                    v[:, 1, :, 0, :], v[:, 1, :, 1, :])

        prev_s = None   # (s_tile, gl) of chunk k-1, awaiting ln/rexp
        prev_nd = None  # (n_tile, dst, gl) of chunk k-1, awaiting out-mult
        out_ctr = [0]

        def emit_recip(st):
            # r = exp(-ln(s)): both fns live in one ACT table set, and the
            # iterative DVE divide would be far slower.  lns stays f32 (a bf16
            # lns would cost ~|lns|*2^-9 ~ 1% relative error after the exp).
            s_t, gl_l = st
            pn_l = s_t.shape[0]
            lns = tmp.tile([pn_l, f // 2], F32, name="lns", tag="lns")
            nc.scalar.activation(lns[:, 0:gl_l], s_t[:, 0:gl_l],
                                 mybir.ActivationFunctionType.Ln)
            r = dfr.tile([pn_l, f // 2], BF16, name="r", tag="r")
            nc.scalar.activation(r[:, 0:gl_l], lns[:, 0:gl_l],
                                 mybir.ActivationFunctionType.Exp, scale=-1.0)
            return r

        def emit_tail(n_t, r_t, dst, gl_l):
            pn_l = n_t.shape[0]
            tag = f"outt{out_ctr[0] % 2}"
            out_ctr[0] += 1
            outt = dfr.tile([pn_l, f // 2], BF16, name=tag, tag=tag)
            nc.vector.tensor_tensor(outt[:, 0:gl_l], n_t[:, 0:gl_l],
                                    r_t[:, 0:gl_l], mul)
            # SWDGE store widens bf16 -> f32 in flight
            nc.gpsimd.dma_start(dst, outt[:, 0:gl_l])

        xin = load(0)
        for k, (bi, off, fl) in enumerate(chunks):
            hp0 = bi * hp_pb
            gl = fl // 2
            # ---- ACT stream: prefetch next, exp, deferred recip, convert
            xin_next = load(k + 1) if k + 1 < len(chunks) else None

            ex = epp.tile([pn, 2 * f], BF16, name="ex", tag="ex")
            nc.scalar.activation(ex[:, 0:2 * fl], xin[:, 0:2 * fl],
                                 mybir.ActivationFunctionType.Exp)
            r = emit_recip(prev_s) if prev_s is not None else None

            xb = epp.tile([pn, f], BF16, name="xb", tag="xb")
            nc.scalar.activation(xb[:, 0:fl], xin[:, 0:fl],
                                 mybir.ActivationFunctionType.Copy)

            # ---- DVE stream
            EA, EB, EC, ED = quad(ex, fl)
            Av, Bv = quad(xb, fl, dq=2)
            _, _, Cv, Dv = quad(xin, fl)

            def t3(tag, dtype=BF16):
                t = tmp.tile([pn, f // 2], dtype, name=tag, tag=tag)
                return t, t[:, 0:gl].rearrange("p (q c) -> p q c",
                                               q=fl // (2 * c), c=c)

            # denominator first: its ln/rexp must be ready one chunk later
            s12, s12v = t3("s12")
            nc.vector.tensor_tensor(s12v, EA, EB, add)
            s34, s34v = t3("s34")
            nc.vector.tensor_tensor(s34v, EC, ED, add)
            s_t = dfr.tile([pn, f // 2], BF16, name="s", tag="s")
            nc.vector.tensor_tensor(
                s_t[:, 0:gl].rearrange("p (q c) -> p q c", q=fl // (2 * c), c=c),
                s12v, s34v, add)

            m3, m3v = t3("m3")
            nc.vector.tensor_tensor(m3v, Cv, EC, mul)
            m4, m4v = t3("m4")
            nc.vector.tensor_tensor(m4v, Dv, ED, mul)

            if prev_nd is not None:
                emit_tail(prev_nd[0], r, prev_nd[1], prev_nd[2])

            m1, m1v = t3("m1")
            nc.vector.tensor_tensor(m1v, Av, EA, mul)
            m2, m2v = t3("m2")
            nc.vector.tensor_tensor(m2v, Bv, EB, mul)
            n12, n12v = t3("n12")
            nc.vector.tensor_tensor(n12v, m1v, m2v, add)
            n34, n34v = t3("n34")
            nc.vector.tensor_tensor(n34v, m3v, m4v, add)
            n_t = dfr.tile([pn, f // 2], BF16, name="n", tag="n")
            nc.vector.tensor_tensor(
                n_t[:, 0:gl].rearrange("p (q c) -> p q c", q=fl // (2 * c), c=c),
                n12v, n34v, add)

            prev_s = (s_t, gl)
            prev_nd = (n_t, yq[hp0:hp0 + hp_pb, :, off // 2:off // 2 + gl], gl)
            xin = xin_next

        # drain: last chunk's recip + tail
        r = emit_recip(prev_s)
        emit_tail(prev_nd[0], r, prev_nd[1], prev_nd[2])

    return _legalize_waits(nc) if legalize else nc


def kernel(**inputs) -> np.ndarray:
    from concourse.bass_utils import run_bass_kernel_spmd

    x = inputs["x"]
    assert x.shape == (B, H, W, C) and x.dtype == np.float32
    nc = build_kernel()
    shards = x.reshape(N_CORES, B_LOC, H, W, C)
    in_maps = [{"x": np.ascontiguousarray(shards[i])} for i in range(N_CORES)]
    res = run_bass_kernel_spmd(nc, in_maps, list(range(N_CORES)))
    return np.concatenate([r["y"] for r in res.results], axis=0)


if __name__ == "__main__":
    # Small-shape CoreSim validation (no hardware).
    from concourse.bass_interp import CoreSim

    b_loc, h, w, c, f = 1, 8, 16, 128, 512
    nc = build_kernel(b_loc, h, w, c, f, legalize=False)
    rng = np.random.default_rng(0)
    xs = rng.standard_normal((b_loc, h, w, c), dtype=np.float32)

    sim = CoreSim(nc)
    sim.tensor("x")[:] = xs
    sim.simulate()
    got = sim.tensor("y").copy()

    xd = xs.astype(np.float64)
    p = xd.reshape(b_loc, h // 2, 2, w // 2, 2, c).transpose(0, 1, 3, 2, 4, 5)
    p = p.reshape(b_loc, h // 2, w // 2, 4, c)
    e = np.exp(p - p.max(axis=3, keepdims=True))
    ref = (p * e).sum(axis=3) / e.sum(axis=3)
    err = np.abs(got - ref).max() / np.abs(ref).max()
    print("scale-rel err:", err, "max abs err:", np.abs(got - ref).max())
    assert err < 2e-2, "sim mismatch"
    print("SIM OK")
